# revision 8
# baseline (speedup 1.0000x reference)
"""Trainium2 Bass kernel for a TF-style GRU + sigmoid projection.

Reference computation (B=32, T=2048, D=H=OUT=256):
    ru  = sigmoid([x_t, h] @ Wg + bg);  r, u = split(ru)
    c   = tanh([x_t, r*h] @ Wc + bc)
    h'  = u*h + (1-u)*c
    out = sigmoid(H @ Wp + bp)          # H = all h_t

Strategy: data-parallel over batch (8 cores x 4 sequences).  Everything on
chip lives "hidden-major" (transposed): tensors are [hidden(128-part) x
(k-tile, time*batch)] so per-step elementwise/activation ops use all 128
lanes.  The x-dependent halves of the gate/candidate matmuls are precomputed
per CHUNK-step chunk directly into PSUM banks; the sequential loop
accumulates the h-dependent matmuls on top (start=False on everything but
the first touch of each bank), so no explicit adds are needed and the
sigmoid/tanh read the finished pre-activations straight out of PSUM.
Projection runs per chunk, interleaved with the recurrence on the idle
slack of the tensor engine.
"""

import numpy as np

B, T, D = 32, 2048, 256
H, OUT = 256, 256
NCORES = 8
BLOC = B // NCORES  # 4 sequences per core
CHUNK = 32          # steps per PSUM staging chunk (each PSUM tile must fit one 2KB bank)

_cache = {}


def _build(T_, C_):
    import concourse.bacc as bacc
    import concourse.mybir as mybir
    from concourse.tile import TileContext

    f32 = mybir.dt.float32
    bf16 = mybir.dt.bfloat16
    AF = mybir.ActivationFunctionType
    ALU = mybir.AluOpType

    TB = T_ * BLOC
    CB = C_ * BLOC
    nchunks = T_ // C_

    nc = bacc.Bacc("TRN2", target_bir_lowering=False, debug=False)

    xT_d = nc.declare_dram_parameter("xT", [2, 128, TB], bf16, isOutput=False)
    wgx_d = nc.declare_dram_parameter("Wgx", [2, 128, 512], bf16, isOutput=False)
    wgh_d = nc.declare_dram_parameter("Wgh", [2, 128, 512], bf16, isOutput=False)
    wcx_d = nc.declare_dram_parameter("Wcx", [2, 128, 256], bf16, isOutput=False)
    wch_d = nc.declare_dram_parameter("Wch", [2, 128, 256], bf16, isOutput=False)
    wp_d = nc.declare_dram_parameter("Wp", [2, 128, 256], bf16, isOutput=False)
    bg_d = nc.declare_dram_parameter("bg", [1, 512], bf16, isOutput=False)
    bc_d = nc.declare_dram_parameter("bc", [1, 256], bf16, isOutput=False)
    bp_d = nc.declare_dram_parameter("bp", [1, 256], bf16, isOutput=False)
    outT_d = nc.declare_dram_parameter("outT", [2, 128, TB], f32, isOutput=True)

    with TileContext(nc) as tc:
        with (
            tc.tile_pool(name="const", bufs=1) as const,
            tc.tile_pool(name="small", bufs=3) as small,
            tc.tile_pool(name="outp", bufs=3) as outp,
            tc.tile_pool(name="psg", bufs=2, space="PSUM") as psg,
            tc.tile_pool(name="psp", bufs=2, space="PSUM") as psp,
        ):
            xT = const.tile([128, 2, TB], bf16)
            hT = const.tile([128, 2, TB], bf16)
            wgx = const.tile([128, 2, 512], bf16)
            wgh = const.tile([128, 2, 512], bf16)
            wcx = const.tile([128, 2, 256], bf16)
            wch = const.tile([128, 2, 256], bf16)
            wp = const.tile([128, 2, 256], bf16)
            bg = const.tile([1, 512], bf16)
            bc = const.tile([1, 256], bf16)
            bp = const.tile([1, 256], bf16)
            ones = const.tile([1, CB], bf16)
            h0b = const.tile([128, 2, BLOC], bf16)

            for k in range(2):
                nc.sync.dma_start(out=xT[:, k, :], in_=xT_d[k])
                nc.sync.dma_start(out=wgx[:, k, :], in_=wgx_d[k])
                nc.sync.dma_start(out=wgh[:, k, :], in_=wgh_d[k])
                nc.sync.dma_start(out=wcx[:, k, :], in_=wcx_d[k])
                nc.sync.dma_start(out=wch[:, k, :], in_=wch_d[k])
                nc.sync.dma_start(out=wp[:, k, :], in_=wp_d[k])
            nc.sync.dma_start(out=bg[:], in_=bg_d[:])
            nc.sync.dma_start(out=bc[:], in_=bc_d[:])
            nc.sync.dma_start(out=bp[:], in_=bp_d[:])
            nc.vector.memset(ones[:], 1.0)
            nc.vector.memset(h0b[:], 0.0)

            def precompute_tiles(c):
                """Fresh PSUM tiles for chunk c (pg holds r|u j-major, pc the
                candidate).  Returns the tiles plus the list of staging
                matmul thunks, which the step loop interleaves."""
                cols = slice(c * CB, (c + 1) * CB)
                pg = psg.tile([128, C_, 4, BLOC], f32, tag="pg")
                pc = psg.tile([128, C_, 2, BLOC], f32, tag="pc")
                thunks = []

                # start=True clears the has_written bits of the WHOLE bank,
                # so use it exactly once per PSUM tile (first touch).
                def stage(dst, gi, w, k, m, start):
                    def run():
                        nc.tensor.matmul(
                            dst[:, :, gi, :],
                            w[:, k, m:m + 128],
                            xT[:, k, cols],
                            start=start,
                            stop=False,
                        )
                    return run

                def stage_bias(dst, gi, brow, m):
                    def run():
                        nc.tensor.matmul(
                            dst[:, :, gi, :],
                            brow[:1, m:m + 128],
                            ones[:1, :],
                            start=False,
                            stop=False,
                        )
                    return run

                for gi in range(4):  # r0 r1 u0 u1 gate tiles
                    for k in range(2):
                        thunks.append(
                            stage(pg, gi, wgx, k, gi * 128, gi == 0 and k == 0)
                        )
                    thunks.append(stage_bias(pg, gi, bg, gi * 128))
                for gi in range(2):  # candidate tiles
                    for k in range(2):
                        thunks.append(
                            stage(pc, gi, wcx, k, gi * 128, gi == 0 and k == 0)
                        )
                    thunks.append(stage_bias(pc, gi, bc, gi * 128))
                return pg, pc, thunks

            def project_thunks(c):
                """Projection of chunk c's hidden states, as thunks."""
                cols = slice(c * CB, (c + 1) * CB)
                thunks = []
                for mo in range(2):
                    pp = psp.tile([128, CB], f32, tag="pp")

                    def run(pp=pp, mo=mo):
                        for k in range(2):
                            nc.tensor.matmul(
                                pp[:],
                                wp[:, k, mo * 128:(mo + 1) * 128],
                                hT[:, k, cols],
                                start=(k == 0),
                                stop=False,
                            )
                        nc.tensor.matmul(
                            pp[:], bp[:1, mo * 128:(mo + 1) * 128], ones[:1, :],
                            start=False, stop=True,
                        )
                        ob = outp.tile([128, CB], f32, tag="ob")
                        nc.scalar.activation(ob[:], pp[:], AF.Sigmoid)
                        nc.sync.dma_start(out=outT_d[mo, :, cols], in_=ob[:])
                    thunks.append(run)
                return thunks

            def step(pg, pc, j, t, h_prev):
                # gate matmuls accumulate W_h @ h onto the precomputed x-part
                for gi in range(4):
                    for k in range(2):
                        nc.tensor.matmul(
                            pg[:, j, gi, :],
                            wgh[:, k, gi * 128:(gi + 1) * 128],
                            h_prev[:, k, :],
                            start=False,
                            stop=(k == 1),
                        )
                ru = small.tile([128, 4, BLOC], f32, tag="ru")
                nc.scalar.activation(ru[:], pg[:, j, :, :], AF.Sigmoid)
                r = ru[:, 0:2, :]
                u = ru[:, 2:4, :]
                rh = small.tile([128, 2, BLOC], bf16, tag="rh")
                nc.vector.tensor_mul(rh[:], r, h_prev[:])
                for gi in range(2):
                    for k in range(2):
                        nc.tensor.matmul(
                            pc[:, j, gi, :],
                            wch[:, k, gi * 128:(gi + 1) * 128],
                            rh[:, k, :],
                            start=False,
                            stop=(k == 1),
                        )
                uh = small.tile([128, 2, BLOC], f32, tag="uh")
                nc.vector.tensor_mul(uh[:], u, h_prev[:])
                v = small.tile([128, 2, BLOC], f32, tag="v")
                nc.vector.tensor_scalar(v[:], u, -1.0, 1.0, ALU.mult, ALU.add)
                c_sb = small.tile([128, 2, BLOC], f32, tag="c")
                nc.scalar.activation(c_sb[:], pc[:, j, :, :], AF.Tanh)
                e = small.tile([128, 2, BLOC], f32, tag="e")
                nc.vector.tensor_mul(e[:], v[:], c_sb[:])
                # h' = e + u*h straight into the bf16 history buffer (it is
                # both the next step's matmul operand and the recurrent state)
                nc.vector.tensor_add(hT[:, :, 4 * t:4 * t + 4], e[:], uh[:])

            h_prev = h0b[:, :, :]
            pg, pc, boot = precompute_tiles(0)
            for th in boot:
                th()
            for c in range(nchunks):
                pending = []
                npg = npc = None
                if c + 1 < nchunks:
                    npg, npc, pending = precompute_tiles(c + 1)
                if c > 0:
                    pending = pending + project_thunks(c - 1)
                for j in range(C_):
                    t = c * C_ + j
                    step(pg, pc, j, t, h_prev)
                    h_prev = hT[:, :, 4 * t:4 * t + 4]
                    # spread staging/projection matmuls across the chunk to
                    # fill tensor-engine slack and avoid boundary bubbles
                    if j < len(pending):
                        pending[j]()
                for th in pending[C_:]:
                    th()
                if npg is not None:
                    pg, pc = npg, npc
            for th in project_thunks(nchunks - 1):
                th()

    nc.finalize()
    return nc


def _get_nc(T_, C_):
    key = (T_, C_)
    if key not in _cache:
        _cache[key] = _build(T_, C_)
    return _cache[key]


def _prep_core_inputs(x_core, Wg, bg, Wc, bc, Wp, bp, T_):
    import ml_dtypes

    bf16 = ml_dtypes.bfloat16

    def cast(a):
        return np.ascontiguousarray(a.astype(bf16))

    # hidden-major x: xT[k, p, t*BLOC + b] = x[b, t, k*128+p]
    xT = np.ascontiguousarray(
        x_core.transpose(2, 1, 0).reshape(2, 128, T_ * BLOC)
    )
    return {
        "xT": cast(xT),
        "Wgx": cast(Wg[:256].reshape(2, 128, 512)),
        "Wgh": cast(Wg[256:].reshape(2, 128, 512)),
        "Wcx": cast(Wc[:256].reshape(2, 128, 256)),
        "Wch": cast(Wc[256:].reshape(2, 128, 256)),
        "Wp": cast(Wp.reshape(2, 128, 256)),
        "bg": cast(bg.reshape(1, 512)),
        "bc": cast(bc.reshape(1, 256)),
        "bp": cast(bp.reshape(1, 256)),
    }


def run_gru(x, Wg, bg, Wc, bc, Wp, bp, T_=None, C_=None, trace=False):
    from concourse.bass_utils import run_bass_kernel_spmd

    T_ = T_ or T
    C_ = C_ or CHUNK
    x = np.asarray(x, dtype=np.float32)
    nc = _get_nc(T_, C_)
    in_maps = []
    for core in range(NCORES):
        x_core = x[core * BLOC:(core + 1) * BLOC]
        in_maps.append(_prep_core_inputs(x_core, Wg, bg, Wc, bc, Wp, bp, T_))
    res = run_bass_kernel_spmd(nc, in_maps, list(range(NCORES)), trace=trace)
    outs = []
    for core in range(NCORES):
        oT = res.results[core]["outT"]  # [2, 128, T*BLOC]
        o = oT.reshape(2, 128, T_, BLOC).transpose(3, 2, 0, 1).reshape(BLOC, T_, OUT)
        outs.append(o)
    full = np.concatenate(outs, axis=0).astype(np.float32)
    return full, res


def kernel(x, Wg, bg, Wc, bc, Wp, bp):
    out, _ = run_gru(
        np.asarray(x), np.asarray(Wg), np.asarray(bg), np.asarray(Wc),
        np.asarray(bc), np.asarray(Wp), np.asarray(bp),
    )
    return out
